# revision 2
# baseline (speedup 1.0000x reference)
"""Masked-loss kernel for nn_MLoss_9715216024200 on 8 Trainium2 NeuronCores.

loss = sum(where(y[...,0]>0.5, (y-x)^2 - a*x^2, 0)) + a*sum(x[...,0]^2)
with x,y f32 (256, 10647, 5); output is a f32 scalar.

Sharding: flatten both tensors to cells (5 contiguous f32 each), pad with
256 zero-cells (mathematically neutral: y0=0 -> mask 0, x=0 -> no bg term),
reshape to (8 cores, 128 partitions, 2662 cells).  Each core streams its
13 MiB at the ~360 GB/s HBM roofline; the three compute engines split the
per-tile work so every engine stays under the per-tile DMA time:

  DVE : m  = bf16(y0 > 0.5)           [cells]
        d  = y - x (bf16 out)         [5*cells]
        dm = d * m (broadcast view)   [5*cells] -> head of dmx
  Pool: xs0 = bf16(sqrt(a)*x0)        [cells]  -> tail of dmx
        xm  = x * m (broadcast view)  [5*cells]
  ACT : acc1[t] = sum(dmx^2) = sum((m*d)^2) + a*sum(x0^2)   (Square+accum)
        acc2[t] = a*sum(xm^2)                                (Square+accum)

m*v^2 == (m*v)^2 because m is 0/1.  The mask is never materialized at
feature width: dm/xm read m through a stride-0 broadcast AP, which trades
DVE's 2x bf16 mode for skipping a full [5*cells] mask-broadcast op.

Tile sizes telescope down at the end (253*8 then 200..26) so that when the
last DMA lands only a tiny dependency chain remains; the final tiles'
reductions run as DVE scalar_tensor_tensor accumulates to skip the ACT
round trip.  Host combines: total = sum(acc1) - sum(acc2), in f64 over
8 cores x 128 partitions x N_TILES tiles.
"""
import sys

for _p in ('/opt/trn_rl_repo',):
    if _p in sys.path:
        sys.path.remove(_p)
    sys.path.insert(0, _p)

import numpy as np

B, C, F = 256, 10647, 5
THRESH = 0.5
ALPHA = 0.1
N_CORES = 8
P = 128
CELLS = B * C                      # 2,725,632
CELLS_PER_PART = 2662              # ceil to 8*128*2662 = 2,725,888
PAD_CELLS = N_CORES * P * CELLS_PER_PART - CELLS   # 256
FD = CELLS_PER_PART * F            # 13310 elems per partition per core

import os as _os
_ts = _os.environ.get('TILE_SIZES', '')
TILE_SIZES = ([int(v) for v in _ts.split(',')] if _ts
              else [253] * 8 + [200, 160, 120, 80, 52, 26])
assert sum(TILE_SIZES) == CELLS_PER_PART
N_TILES = len(TILE_SIZES)
# last k tiles: both reductions on DVE via scalar_tensor_tensor
TAIL_DVE = int(_os.environ.get('TAIL_DVE', '1'))
BUFS = [int(v) for v in _os.environ.get('BUFS', '6,6,6,4').split(',')]

_compiled = None


def _build():
    from contextlib import ExitStack
    import concourse.tile as tile
    from concourse import bacc, mybir

    sqa = float(np.sqrt(ALPHA))

    nc = bacc.Bacc("TRN2", target_bir_lowering=False, debug=False,
                   enable_asserts=True, num_devices=N_CORES)
    x_d = nc.dram_tensor("x", [P, FD], mybir.dt.float32, kind="ExternalInput").ap()
    y_d = nc.dram_tensor("y", [P, FD], mybir.dt.float32, kind="ExternalInput").ap()
    o_d = nc.dram_tensor("o", [P, 2 * N_TILES], mybir.dt.float32,
                         kind="ExternalOutput").ap()

    f32 = mybir.dt.float32
    bf16 = mybir.dt.bfloat16
    Sq = mybir.ActivationFunctionType.Square
    Alu = mybir.AluOpType

    with tile.TileContext(nc) as tc, ExitStack() as ctx:
        xp = ctx.enter_context(tc.tile_pool(name="x", bufs=BUFS[0]))
        yp = ctx.enter_context(tc.tile_pool(name="y", bufs=BUFS[1]))
        wp = ctx.enter_context(tc.tile_pool(name="work", bufs=BUFS[2]))
        sp = ctx.enter_context(tc.tile_pool(name="scratch", bufs=BUFS[3]))
        ap_ = ctx.enter_context(tc.tile_pool(name="acc", bufs=1))

        # interleaved acc layout: columns [2t, 2t+1] = (dm-side, xm-side)
        acc = ap_.tile([P, 2 * N_TILES], f32)

        tail = []
        off = 0
        for t, cells in enumerate(TILE_SIZES):
            fd = cells * F
            yt = yp.tile([P, fd], f32, tag="yt")
            xt = xp.tile([P, fd], f32, tag="xt")
            sl = slice(off, off + fd)
            off += fd
            nc.sync.dma_start(yt[:], y_d[:, sl])
            nc.sync.dma_start(xt[:], x_d[:, sl])

            # DVE: per-cell mask (bf16 0/1), read by dm/xm as a broadcast view
            m = wp.tile([P, cells], bf16, tag="m")
            nc.vector.tensor_scalar(m[:], yt[:, 0::F], THRESH, None,
                                    op0=Alu.is_gt)
            mv = m[:].unsqueeze(2).broadcast_to((P, cells, F))

            # dmx = [dm (fd) | xs0 (cells)]: one fused Square+accum covers
            # sum((m*d)^2) + a*sum(x0^2)
            dmx = wp.tile([P, fd + cells], bf16, tag="dmx")

            dt_ = wp.tile([P, fd], bf16, tag="d")
            nc.vector.tensor_tensor(dt_[:], yt[:], xt[:], op=Alu.subtract)
            nc.vector.tensor_tensor(
                dmx[:, 0:fd].rearrange("p (k f) -> p k f", f=F),
                dt_[:].rearrange("p (k f) -> p k f", f=F), mv, op=Alu.mult)

            # Pool: xs0 then xm (xs0 first: it gates the dmx reduction)
            nc.gpsimd.tensor_scalar(dmx[:, fd:fd + cells], xt[:, 0::F],
                                    sqa, None, op0=Alu.mult)
            xmt = wp.tile([P, fd], bf16, tag="xm")
            nc.gpsimd.tensor_tensor(
                xmt[:].rearrange("p (k f) -> p k f", f=F),
                xt[:].rearrange("p (k f) -> p k f", f=F), mv, op=Alu.mult)

            if t >= N_TILES - TAIL_DVE:
                tail.append((t, dmx, xmt, cells))
            else:
                sq = sp.tile([P, fd + cells], bf16, tag="sq")
                nc.scalar.activation(sq[:], dmx[:], Sq,
                                     accum_out=acc[:, 2 * t:2 * t + 1])
                sq2 = sp.tile([P, fd], bf16, tag="sq2")
                nc.scalar.activation(sq2[:], xmt[:], Sq, scale=sqa,
                                     accum_out=acc[:, 2 * t + 1:2 * t + 2])

        for (t, dmx, xmt, cells) in tail:
            # deferred past the loop so the last tiles' d/dm (which gate
            # these) run first on DVE
            fd = cells * F
            s1 = sp.tile([P, fd + cells], bf16, tag="sq")
            nc.vector.scalar_tensor_tensor(
                s1[:], dmx[:], 1.0, dmx[:],
                op0=Alu.mult, op1=Alu.mult, accum_out=acc[:, 2 * t:2 * t + 1])
            s2 = sp.tile([P, fd], bf16, tag="sq2")
            nc.vector.scalar_tensor_tensor(
                s2[:], xmt[:], ALPHA, xmt[:],
                op0=Alu.mult, op1=Alu.mult,
                accum_out=acc[:, 2 * t + 1:2 * t + 2])

        nc.sync.dma_start(o_d[:], acc[:])

    nc.compile()
    return nc


def _shard(a: np.ndarray) -> list[np.ndarray]:
    flat = a.reshape(-1)
    pad = np.zeros(PAD_CELLS * F, dtype=a.dtype)
    flat = np.concatenate([flat, pad])
    per_core = flat.reshape(N_CORES, P, FD)
    return [np.ascontiguousarray(per_core[i]) for i in range(N_CORES)]


def kernel(x: np.ndarray, y: np.ndarray) -> np.ndarray:
    global _compiled
    if _compiled is None:
        _compiled = _build()
    nc = _compiled

    from concourse.bass_utils import run_bass_kernel_spmd

    xs = _shard(np.asarray(x, dtype=np.float32))
    ys = _shard(np.asarray(y, dtype=np.float32))
    in_maps = [{"x": xs[i], "y": ys[i]} for i in range(N_CORES)]
    res = run_bass_kernel_spmd(nc, in_maps, core_ids=list(range(N_CORES)))

    total = np.float64(0.0)
    for r in res.results:
        o = r["o"].astype(np.float64)
        total += o[:, 0::2].sum()
        total -= o[:, 1::2].sum()
    return np.float32(total)


# revision 3
# speedup vs baseline: 1.0135x; 1.0135x over previous
"""Masked-loss kernel for nn_MLoss_9715216024200 on 8 Trainium2 NeuronCores.

loss = sum(where(y[...,0]>0.5, (y-x)^2 - a*x^2, 0)) + a*sum(x[...,0]^2)
with x,y f32 (256, 10647, 5); output is a f32 scalar.

Sharding: flatten both tensors to cells (5 contiguous f32 each), pad with
256 zero-cells (mathematically neutral: y0=0 -> mask 0, x=0 -> no bg term),
reshape to (8 cores, 128 partitions, 2662 cells).  Each core streams its
13 MiB at the ~360 GB/s HBM roofline; the three compute engines split the
per-tile work so every engine stays well under the per-tile DMA time:

  DVE : m  = bf16(y0 > 0.5)                    [cells]
        xm = x * m (stride-0 broadcast of m)   [5*cells]
        dm = ym - xm (bf16 2x mode)            [5*cells] -> head of dmx
  Pool: ym  = y * m (broadcast)                [5*cells]
        xs0 = bf16(sqrt(a)*x0)                 [cells]   -> tail of dmx
  ACT : acc1[t] = sum(dmx^2) = sum((m*(y-x))^2) + a*sum(x0^2)
        acc2[t] = sum(xm^2)                    (no scale: host applies a)

m*(y-x) == y*m - x*m and m*v^2 == (m*v)^2 because m is 0/1.  The mask is
read through a stride-0 broadcast AP (never materialized at feature
width); ym/xm land packed bf16 so the subtract runs in DVE 2x mode.

The tile sizes telescope down at the end so only a tiny dependency chain
remains after the last DMA; the last TAIL_DVE tiles' reductions run as
DVE scalar_tensor_tensor accumulates (no ACT accumulator-read latency),
with ym also moved to DVE so the endgame never waits on Pool.

Host combines: total = sum(acc1) - ALPHA*sum(acc2), in f64 over
8 cores x 128 partitions x N_TILES tiles.
"""
import sys

for _p in ('/opt/trn_rl_repo',):
    if _p in sys.path:
        sys.path.remove(_p)
    sys.path.insert(0, _p)

import numpy as np

B, C, F = 256, 10647, 5
THRESH = 0.5
ALPHA = 0.1
N_CORES = 8
P = 128
CELLS = B * C                      # 2,725,632
CELLS_PER_PART = 2662              # ceil to 8*128*2662 = 2,725,888
PAD_CELLS = N_CORES * P * CELLS_PER_PART - CELLS   # 256
FD = CELLS_PER_PART * F            # 13310 elems per partition per core

import os as _os
_ts = _os.environ.get('TILE_SIZES', '')
TILE_SIZES = ([int(v) for v in _ts.split(',')] if _ts
              else [404] * 5 + [238, 160, 100, 70, 48, 26])
assert sum(TILE_SIZES) == CELLS_PER_PART
N_TILES = len(TILE_SIZES)
# last k tiles: both reductions on DVE stt, ym on DVE too
TAIL_DVE = int(_os.environ.get('TAIL_DVE', '2'))
BUFS = [int(v) for v in _os.environ.get('BUFS', '4,4,4,3').split(',')]

_compiled = None


def _build():
    from contextlib import ExitStack
    import concourse.tile as tile
    from concourse import bacc, mybir

    sqa = float(np.sqrt(ALPHA))

    nc = bacc.Bacc("TRN2", target_bir_lowering=False, debug=False,
                   enable_asserts=True, num_devices=N_CORES)
    x_d = nc.dram_tensor("x", [P, FD], mybir.dt.float32, kind="ExternalInput").ap()
    y_d = nc.dram_tensor("y", [P, FD], mybir.dt.float32, kind="ExternalInput").ap()
    o_d = nc.dram_tensor("o", [P, 2 * N_TILES], mybir.dt.float32,
                         kind="ExternalOutput").ap()

    f32 = mybir.dt.float32
    bf16 = mybir.dt.bfloat16
    Sq = mybir.ActivationFunctionType.Square
    Alu = mybir.AluOpType

    with tile.TileContext(nc) as tc, ExitStack() as ctx:
        xp = ctx.enter_context(tc.tile_pool(name="x", bufs=BUFS[0]))
        yp = ctx.enter_context(tc.tile_pool(name="y", bufs=BUFS[1]))
        wp = ctx.enter_context(tc.tile_pool(name="work", bufs=BUFS[2]))
        sp = ctx.enter_context(tc.tile_pool(name="scratch", bufs=BUFS[3]))
        ap_ = ctx.enter_context(tc.tile_pool(name="acc", bufs=1))

        # interleaved acc layout: columns [2t, 2t+1] = (dm-side, xm-side)
        acc = ap_.tile([P, 2 * N_TILES], f32)

        tail = []
        off = 0
        for t, cells in enumerate(TILE_SIZES):
            fd = cells * F
            is_tail = t >= N_TILES - TAIL_DVE
            yt = yp.tile([P, fd], f32, tag="yt")
            xt = xp.tile([P, fd], f32, tag="xt")
            sl = slice(off, off + fd)
            off += fd
            nc.sync.dma_start(yt[:], y_d[:, sl])
            nc.sync.dma_start(xt[:], x_d[:, sl])

            # DVE: per-cell mask (bf16 0/1), read by ym/xm as broadcast view
            m = wp.tile([P, cells], bf16, tag="m")
            nc.vector.tensor_scalar(m[:], yt[:, 0::F], THRESH, None,
                                    op0=Alu.is_gt)
            mv = m[:].unsqueeze(2).broadcast_to((P, cells, F))

            # dmx = [dm (fd) | xs0 (cells)]: one fused Square+accum covers
            # sum((m*(y-x))^2) + a*sum(x0^2)
            dmx = wp.tile([P, fd + cells], bf16, tag="dmx")

            ym = wp.tile([P, fd], bf16, tag="ym")
            (nc.vector if is_tail else nc.gpsimd).tensor_tensor(
                ym[:].rearrange("p (k f) -> p k f", f=F),
                yt[:].rearrange("p (k f) -> p k f", f=F), mv, op=Alu.mult)
            xmt = wp.tile([P, fd], bf16, tag="xm")
            nc.vector.tensor_tensor(
                xmt[:].rearrange("p (k f) -> p k f", f=F),
                xt[:].rearrange("p (k f) -> p k f", f=F), mv, op=Alu.mult)
            # packed bf16 in/out -> DVE 2x mode
            nc.vector.tensor_tensor(dmx[:, 0:fd], ym[:], xmt[:],
                                    op=Alu.subtract)

            # Pool: xs0 into the dmx tail (gates the dmx reduction)
            nc.gpsimd.tensor_scalar(dmx[:, fd:fd + cells], xt[:, 0::F],
                                    sqa, None, op0=Alu.mult)

            if is_tail:
                tail.append((t, dmx, xmt, cells))
            else:
                sq = sp.tile([P, fd + cells], bf16, tag="sq")
                nc.scalar.activation(sq[:], dmx[:], Sq,
                                     accum_out=acc[:, 2 * t:2 * t + 1])
                sq2 = sp.tile([P, fd], bf16, tag="sq2")
                nc.scalar.activation(sq2[:], xmt[:], Sq,
                                     accum_out=acc[:, 2 * t + 1:2 * t + 2])

        for (t, dmx, xmt, cells) in tail:
            # deferred past the loop so the last tiles' ym/xm/dm (which gate
            # these) run first on DVE
            fd = cells * F
            s1 = sp.tile([P, fd + cells], bf16, tag="sq")
            nc.vector.scalar_tensor_tensor(
                s1[:], dmx[:], 1.0, dmx[:],
                op0=Alu.mult, op1=Alu.mult, accum_out=acc[:, 2 * t:2 * t + 1])
            s2 = sp.tile([P, fd], bf16, tag="sq2")
            nc.vector.scalar_tensor_tensor(
                s2[:], xmt[:], 1.0, xmt[:],
                op0=Alu.mult, op1=Alu.mult,
                accum_out=acc[:, 2 * t + 1:2 * t + 2])

        nc.sync.dma_start(o_d[:], acc[:])

    nc.compile()
    return nc


def _shard(a: np.ndarray) -> list[np.ndarray]:
    flat = a.reshape(-1)
    pad = np.zeros(PAD_CELLS * F, dtype=a.dtype)
    flat = np.concatenate([flat, pad])
    per_core = flat.reshape(N_CORES, P, FD)
    return [np.ascontiguousarray(per_core[i]) for i in range(N_CORES)]


def kernel(x: np.ndarray, y: np.ndarray) -> np.ndarray:
    global _compiled
    if _compiled is None:
        _compiled = _build()
    nc = _compiled

    from concourse.bass_utils import run_bass_kernel_spmd

    xs = _shard(np.asarray(x, dtype=np.float32))
    ys = _shard(np.asarray(y, dtype=np.float32))
    in_maps = [{"x": xs[i], "y": ys[i]} for i in range(N_CORES)]
    res = run_bass_kernel_spmd(nc, in_maps, core_ids=list(range(N_CORES)))

    total = np.float64(0.0)
    for r in res.results:
        o = r["o"].astype(np.float64)
        total += o[:, 0::2].sum()
        total -= ALPHA * o[:, 1::2].sum()
    return np.float32(total)
